# revision 6
# baseline (speedup 1.0000x reference)
"""LDA head (segment-reduce + Mahalanobis scores) on 8 Trainium2 NeuronCores.

Strategy (single SPMD NEFF on 8 cores):
  - Stats are class-sharded: core k owns classes [125k, 125k+125) and scans the
    full batch, computing [S1^T | S2^T | counts] = onehot^T @ [z | z^2 | 1] with
    16 accumulating PE matmuls (one per 128-row batch tile).
  - The per-class block [mean^T; log prior; pooled-var partial] is transposed
    into a (128,128) SBUF block and exchanged peer-to-peer with
    remote_dma_broadcast (SBUF->SBUF SWDGE descriptors, no CC-core collective):
    call j sends the block to core me^j's gather slot j.  A
    bir_kernel_barrier_wait (1-byte prelude AllGather, overlapped with phase A)
    gates the trigger so no write can land before a peer's semaphore reset.
    The exchange and its semaphore waits sit BETWEEN two TileContexts: the tile
    scheduler's single-core sim cannot model remotely-incremented semaphores
    (it would report a deadlock), so that code is raw bass.
  - Scores are batch-sharded: core k computes output rows [256k, 256k+256) as
    out = [z^T; 1; 1]^T @ [prec*mean; log prior; -0.5*r]  (+ -0.5*q_b fused into
    the PSUM->SBUF copy as a per-partition activation bias).
  - Because of the XOR addressing, core r's gather slot j holds class chunk
    r^j; the host un-permutes the 8 column blocks of each core's output.

kernel(z, y) takes the full inputs and returns the full (2048, 1000) output.
"""

import sys
import numpy as np

if "/opt/trn_rl_repo" not in sys.path:
    sys.path.insert(0, "/opt/trn_rl_repo")

import concourse.bacc as bacc
import concourse.bass as bass
import concourse.mybir as mybir
from concourse import tile
from concourse.bass_utils import run_bass_kernel_spmd

B, C, D = 2048, 1000, 64
NCORES = 8
CL = C // NCORES            # 125 classes per core
NT = B // 128               # 16 batch tiles
BL = B // NCORES            # 256 output rows per core
JT = BL // 128              # 2 local batch tiles
EPS_STATS = 1e-5
EPS_PREC = 1e-6
TSUM = float(np.float32(B) + np.float32(C * EPS_STATS))   # counts.sum()
FP = mybir.dt.float32
AF = mybir.ActivationFunctionType
ALU = mybir.AluOpType


def build_program():
    nc = bacc.Bacc("TRN2", target_bir_lowering=False, debug=False,
                   num_devices=NCORES)

    z_in = nc.dram_tensor("z_in", [B, D], FP, kind="ExternalInput")
    ycols = nc.dram_tensor("ycols", [128, NT], FP, kind="ExternalInput")
    cvals = nc.dram_tensor("cvals", [128, CL], FP, kind="ExternalInput")
    zloc = nc.dram_tensor("zloc", [128, JT, D], FP, kind="ExternalInput")
    ident = nc.dram_tensor("ident", [128, 128], FP, kind="ExternalInput")
    out = nc.dram_tensor("out_loc", [BL, C], FP, kind="ExternalOutput")

    # remote-exchange semaphores (kernel range: zeroed by NRT each execution)
    rsem = nc.alloc_semaphore("gather_rsem")
    lsem = nc.alloc_semaphore("gather_lsem")
    psem = nc.alloc_semaphore("gather_psem")

    # statics that must survive across the two tile contexts
    gat = nc.alloc_sbuf_tensor("gatherbuf", [128, NCORES, 128], FP)
    G = nc.alloc_sbuf_tensor("gblock", [128, 128], FP)
    zTq = nc.alloc_sbuf_tensor("ztq_sb", [65, 256], FP)
    zsq = nc.alloc_sbuf_tensor("zsq_sb", [128, JT, D], FP)
    idn = nc.alloc_sbuf_tensor("idn_sb", [128, 128], FP)
    qsb = nc.alloc_sbuf_tensor("qsb_sb", [128, JT], FP)

    # ---- tile context 1: class-sharded segment stats ------------------------
    with tile.TileContext(nc) as tc:
        with tc.tile_pool(name="sb", bufs=1) as pool, \
             tc.tile_pool(name="ps", bufs=8, space="PSUM") as pp:

            cv = pool.tile([128, CL], FP)
            nc.sync.dma_start(cv[:], cvals[:, :])
            yc = pool.tile([128, NT], FP)
            nc.sync.dma_start(yc[:], ycols[:, :])
            nc.sync.dma_start(idn[:, :], ident[:, :])
            zl = pool.tile([128, JT, D], FP)
            nc.sync.dma_start(zl[:], zloc[:, :, :])
            M = pool.tile([128, NT, 130], FP)      # [z | z^2 | 1 | pad]
            # batch row p*NT+t -> partition p: one contiguous 4KB run/partition
            nc.sync.dma_start(M[:, :, 0:D],
                              z_in[:, :].rearrange("(p t) d -> p t d", p=128))

            # z^2 on DVE so phase A does not wait on scalar act-table loads
            nc.vector.tensor_tensor(M[:, :, D:2 * D], M[:, :, 0:D],
                                    M[:, :, 0:D], ALU.mult)
            nc.vector.memset(M[:, :, 2 * D:2 * D + 1], 1.0)

            # all on DVE: gpsimd shares SBUF ports with DVE and slows
            # these ops ~7x when run concurrently
            oh = pool.tile([128, NT, CL], FP)
            for t in range(NT):
                nc.vector.tensor_scalar(oh[:, t, :], cv[:], yc[:, t:t + 1],
                                        None, ALU.is_equal)

            psS = pp.tile([CL, 129], FP, tag="ps")
            for t in range(NT):
                nc.tensor.matmul(psS[:], lhsT=oh[:, t, :], rhs=M[:, t, 0:129],
                                 start=(t == 0), stop=(t == NT - 1))

            # stats post-processing, all in class-partition layout
            cnt = pool.tile([CL, 1], FP)
            nc.vector.tensor_scalar_add(cnt[:], psS[:, 128:129], EPS_STATS)
            rcp = pool.tile([CL, 1], FP)
            nc.vector.reciprocal(rcp[:], cnt[:])

            TB = pool.tile([CL, 97], FP)           # [mean^T | p-col | beta]
            nc.vector.memset(TB[:, :], 0.0)
            nc.vector.tensor_scalar(TB[:, 0:64], psS[:, 0:64], rcp[:], None,
                                    ALU.mult)
            nc.scalar.activation(TB[:, 96:97], cnt[:], AF.Ln,
                                 scale=1.0 / TSUM)

            cnt2 = pool.tile([CL, 1], FP)
            nc.vector.tensor_scalar_add(cnt2[:], psS[:, 128:129],
                                        2.0 * EPS_STATS)
            rcp2 = pool.tile([CL, 1], FP)
            nc.vector.tensor_tensor(rcp2[:], rcp[:], rcp[:], ALU.mult)
            alph = pool.tile([CL, 1], FP)
            nc.vector.tensor_tensor(alph[:], cnt2[:], rcp2[:], ALU.mult)

            s1sq = pool.tile([CL, 64], FP)
            nc.scalar.activation(s1sq[:], psS[:, 0:64], AF.Square)
            t1 = pool.tile([CL, 64], FP)
            nc.vector.tensor_scalar(t1[:], s1sq[:], alph[:], None, ALU.mult)
            ptile = pool.tile([CL, 64], FP)
            nc.vector.tensor_tensor(ptile[:], psS[:, 64:128], t1[:],
                                    ALU.subtract)

            onesc = pool.tile([CL, 1], FP)
            nc.vector.memset(onesc[:], 1.0)
            psP = pp.tile([64, 1], FP, tag="ps")
            nc.tensor.matmul(psP[:], lhsT=ptile[:], rhs=onesc[:],
                             start=True, stop=True)
            nc.scalar.copy(TB[0:64, 64:65], psP[:])

            # transpose to [97 rows, 125 classes], padded to a (128,128) block
            psT = pp.tile([97, 125], FP, tag="ps")
            nc.tensor.transpose(psT[:], TB[:, :], idn[0:CL, 0:CL])
            nc.vector.memset(G[:, :], 0.0)
            nc.scalar.copy(G[0:97, 0:125], psT[:])

            # local z^T for the score matmuls
            nc.vector.memset(zTq[64:65, :], 1.0)
            for j in range(JT):
                psZ = pp.tile([64, 128], FP, tag="ps")
                nc.tensor.transpose(psZ[:], zl[:, j, :], idn[:, :])
                nc.scalar.copy(zTq[0:64, j * 128:(j + 1) * 128], psZ[:])
            nc.scalar.activation(zsq[:, :, :], zl[:], AF.Square)

    # ---- raw bass: peer-to-peer all-gather via remote SBUF writes -----------
    # (outside the tile contexts: the scheduler's sim cannot model sems that
    # are incremented by other cores)
    for j in range(NCORES):
        rdests = [None] * NCORES
        rdests[j] = (0, j)                         # XOR-relative: dest = me^j
        nc.gpsimd.remote_dma_broadcast(
            gat[:, j, :], G[:, :],
            remote_sem=rsem, local_sem=lsem, rdests=rdests).then_inc(psem, 1)
    # all peers past their semaphore reset before any data flies
    nc.gpsimd.bir_kernel_barrier_wait([list(range(NCORES))])
    nc.gpsimd.wait_ge(psem, NCORES)                # descgen committed
    nc.gpsimd.trigger_dma(count=NCORES)
    # data arrival gates for every consumer engine (8 senders x 2 each)
    nc.vector.wait_ge(rsem, 16)
    nc.scalar.wait_ge(rsem, 16)
    nc.tensor.wait_ge(rsem, 16)
    nc.gpsimd.wait_ge(rsem, 16)
    nc.sync.wait_ge(rsem, 16)

    # ---- tile context 2: batch-sharded Mahalanobis scores -------------------
    with tile.TileContext(nc) as tc2:
        with tc2.tile_pool(name="sb2", bufs=1) as pool, \
             tc2.tile_pool(name="ps2", bufs=8, space="PSUM") as pp:

            mview = gat[0:64, :, 0:CL]             # mean^T  [64, 8, 125]

            # pooled variance: sum the 8 partial vectors on gather row 64
            ptot = pool.tile([1, 64], FP)
            nc.vector.reduce_sum(
                ptot[:], gat[64:65, :, 0:64].rearrange("o j d -> o d j"),
                axis=mybir.AxisListType.X)
            pooled = pool.tile([1, 64], FP)
            nc.vector.tensor_scalar(pooled[:], ptot[:], 1.0 / TSUM,
                                    EPS_STATS, ALU.mult, ALU.add)
            pmax = pool.tile([1, 64], FP)
            nc.vector.tensor_scalar_max(pmax[:], pooled[:], EPS_PREC)
            prow = pool.tile([1, 64], FP)
            nc.vector.reciprocal(prow[:], pmax[:])

            # mean^2 is prec-independent: runs right after the gather lands
            msq = pool.tile([64, C], FP)
            nc.scalar.activation(
                msq[:].rearrange("d (j c) -> d j c", j=NCORES), mview,
                AF.Square)

            # PE warm-up (HAM needs ~3.4us busy before the score matmuls)
            junkps = pp.tile([64, 64], FP, tag="ps")
            for w in range(10):
                nc.tensor.matmul(junkps[:], lhsT=idn[0:64, 0:64],
                                 rhs=gat[0:64, w % NCORES, 0:64],
                                 start=True, stop=True)

            # prec as a column (for V scale + r matmul) and broadcast rows
            psPC = pp.tile([64, 1], FP, tag="ps")
            nc.tensor.transpose(psPC[:], prow[:], idn[0:1, 0:1])
            prec = pool.tile([64, 1], FP)
            nc.vector.tensor_copy(prec[:], psPC[:])
            onesr = pool.tile([1, 128], FP)
            nc.vector.memset(onesr[:], 1.0)
            psPB = pp.tile([128, 64], FP, tag="ps")
            nc.tensor.matmul(psPB[:], lhsT=onesr[:], rhs=prow[:],
                             start=True, stop=True)
            precbc = pool.tile([128, 64], FP)
            nc.vector.tensor_copy(precbc[:], psPB[:])

            V = pool.tile([65, C], FP)              # [prec*mean; beta - r/2]
            nc.vector.tensor_scalar(
                V[0:64, :].rearrange("d (j c) -> d j c", j=NCORES), mview,
                prec[:], None, ALU.mult)
            # combined last row beta - r/2, built at base partition 0
            # (TensorTensor requires equal SB base partitions for its inputs)
            bstage = pool.tile([1, C], FP)
            nc.scalar.copy(
                bstage[:].rearrange("o (j c) -> o j c", j=NCORES),
                gat[96:97, :, 0:CL])
            rrow = pool.tile([1, C], FP)
            for h in range(2):
                psR = pp.tile([1, 500], FP, tag="ps")
                nc.tensor.matmul(psR[:], lhsT=prec[:],
                                 rhs=msq[:, h * 500:(h + 1) * 500],
                                 start=True, stop=True)
                nc.scalar.activation(rrow[:, h * 500:(h + 1) * 500], psR[:],
                                     AF.Copy, scale=-0.5)
            nc.vector.tensor_tensor(rrow[:], rrow[:], bstage[:], ALU.add)
            nc.scalar.copy(V[64:65, :], rrow[:])

            junk = pool.tile([128, 64], FP)
            qraw = pool.tile([128, JT], FP)
            for j in range(JT):
                nc.vector.tensor_tensor(junk[:], zsq[:, j, :], precbc[:],
                                        ALU.mult)
                nc.vector.reduce_sum(qraw[:, j:j + 1], junk[:],
                                     axis=mybir.AxisListType.X)
            nc.vector.tensor_scalar_mul(qsb[:, :], qraw[:], -0.5)

            for j in range(JT):
                outj = pool.tile([128, C], FP, tag=f"outsb{j}")
                for h in range(2):
                    psO = pp.tile([128, 500], FP, tag="ps")
                    nc.tensor.matmul(psO[:],
                                     lhsT=zTq[:, j * 128:(j + 1) * 128],
                                     rhs=V[:, h * 500:(h + 1) * 500],
                                     start=True, stop=True)
                    nc.scalar.activation(outj[:, h * 500:(h + 1) * 500],
                                         psO[:], AF.Identity,
                                         bias=qsb[:, j:j + 1], scale=1.0)
                nc.sync.dma_start(out[j * 128:(j + 1) * 128, :], outj[:])

    nc.compile()
    return nc


_NC_CACHE = None


def _get_program():
    global _NC_CACHE
    if _NC_CACHE is None:
        _NC_CACHE = build_program()
    return _NC_CACHE


def make_in_maps(z, y):
    z = np.ascontiguousarray(np.asarray(z, dtype=np.float32))
    yf = np.asarray(y).astype(np.float32)          # labels < 1000, exact
    ycols_np = np.ascontiguousarray(yf.reshape(128, NT))
    ident_np = np.eye(128, dtype=np.float32)
    in_maps = []
    for k in range(NCORES):
        cvals_np = np.broadcast_to(
            np.arange(k * CL, (k + 1) * CL, dtype=np.float32), (128, CL))
        zloc_np = np.ascontiguousarray(
            z[k * BL:(k + 1) * BL].reshape(JT, 128, D).transpose(1, 0, 2))
        in_maps.append({
            "z_in": z,
            "ycols": ycols_np,
            "cvals": np.ascontiguousarray(cvals_np),
            "zloc": zloc_np,
            "ident": ident_np,
        })
    return in_maps


def run(z, y, trace=False, **kwargs):
    nc = _get_program()
    res = run_bass_kernel_spmd(nc, make_in_maps(z, y), list(range(NCORES)),
                               trace=trace, **kwargs)
    # core r's column block j holds class chunk r^j (j<4) / r^j^2 (j>=4):
    # the broadcast ucode's D2D slot group {4..7} lands with an extra XOR 2
    blocks = []
    for r in range(NCORES):
        loc = res.results[r]["out_loc"].reshape(BL, NCORES, CL)
        perm = [(r ^ c) if (r ^ c) < 4 else (r ^ c) ^ 2 for c in range(NCORES)]
        blocks.append(loc[:, perm, :].reshape(BL, C))
    full = np.concatenate(blocks, axis=0)
    return full, res


def kernel(z, y):
    full, _ = run(z, y, trace=False)
    return full


if __name__ == "__main__":
    rng = np.random.default_rng(0)
    z = rng.standard_normal((B, D), dtype=np.float32)
    y = rng.integers(0, C, size=(B,)).astype(np.int64)
    out = kernel(z, y)
    print("out", out.shape, out.dtype, out[0, :4])


# revision 12
# speedup vs baseline: 2.9583x; 2.9583x over previous
"""LDA head (segment-reduce + Mahalanobis scores) on 8 Trainium2 NeuronCores.

Strategy (single SPMD NEFF on 8 cores, fully replicated stats — no
cross-core communication):
  - Every core reads the full batch and computes the segment stats for ALL
    1000 classes:  psSS = M^T @ onehot  with M = [z | z^2] in fp16, i.e. 16
    accumulating PE matmuls (one per 128-row batch tile) producing
    [S1^T; S2^T] = (128, 1000) fp32 in PSUM — already in the d-partition
    layout the score phase needs, so no transposes and no collective at all.
    (The previous designs exchanged class-sharded stats via a CC AllGather /
    remote SBUF DMAs; both cost 50-150us in collective machinery and launch-
    skew barrier waits.  Recomputing on every core costs ~15us of fp16 PE.)
  - Class counts come from a ones^T @ ohsum matmul (ohsum = sum of the 16
    one-hot tiles on DVE).  fp16 is exact for the one-hot compare (labels
    < 2048) and for the counts; z in fp16 only affects mean/var by ~1e-3
    relative, far inside the 2e-2 gate.
  - Per-column (per-class) scaling rows (1/counts, count correction) are
    broadcast across partitions with ones^T @ row PE matmuls.
  - Scores are batch-sharded: core k computes output rows [256k, 256k+256) as
    out = [z^T; 1]^T @ [prec*mean; log prior - 0.5*r]  (+ -0.5*q_b fused into
    the PSUM->SBUF copy as a per-partition activation bias), with z in fp32.

kernel(z, y) takes the full inputs and returns the full (2048, 1000) output.
"""

import sys
import numpy as np

if "/opt/trn_rl_repo" not in sys.path:
    sys.path.insert(0, "/opt/trn_rl_repo")

import concourse.bacc as bacc
import concourse.bass as bass
import concourse.mybir as mybir
from concourse import tile
from concourse.bass_utils import run_bass_kernel_spmd

B, C, D = 2048, 1000, 64
NCORES = 8
NT = B // 128               # 16 batch tiles
BL = B // NCORES            # 256 output rows per core
JT = BL // 128              # 2 local batch tiles
CH = C // 2                 # 500-column halves (PSUM bank = 2KB)
EPS_STATS = 1e-5
EPS_PREC = 1e-6
TSUM = float(np.float32(B) + np.float32(C * EPS_STATS))   # counts.sum()
FP = mybir.dt.float32
FH = mybir.dt.float16
AF = mybir.ActivationFunctionType
ALU = mybir.AluOpType


def build_program():
    nc = bacc.Bacc("TRN2", target_bir_lowering=False, debug=False,
                   num_devices=NCORES)

    z_in = nc.dram_tensor("z_in", [B, D], FP, kind="ExternalInput")
    ycols = nc.dram_tensor("ycols", [128, NT], FP, kind="ExternalInput")
    cvals = nc.dram_tensor("cvals", [128, C], FH, kind="ExternalInput")
    zloc = nc.dram_tensor("zloc", [128, JT, D], FP, kind="ExternalInput")
    ident = nc.dram_tensor("ident", [128, 128], FP, kind="ExternalInput")
    out = nc.dram_tensor("out_loc", [BL, C], FP, kind="ExternalOutput")

    with tile.TileContext(nc) as tc:
        with tc.tile_pool(name="sb", bufs=1) as pool, \
             tc.tile_pool(name="pss", bufs=1, space="PSUM") as ppS, \
             tc.tile_pool(name="ps", bufs=6, space="PSUM") as pp:

            # ---- input DMAs -------------------------------------------------
            cv = pool.tile([128, C], FH)
            nc.sync.dma_start(cv[:], cvals[:, :])
            yc = pool.tile([128, NT], FP)
            nc.sync.dma_start(yc[:], ycols[:, :])
            idn = pool.tile([128, 128], FP)
            nc.sync.dma_start(idn[:], ident[:, :])
            zl = pool.tile([128, JT, D], FP)
            nc.sync.dma_start(zl[:], zloc[:, :, :])
            Mf = pool.tile([128, NT, D], FP)
            # batch row p*NT+t -> partition p: one contiguous 4KB run/partition
            nc.sync.dma_start(Mf[:, :, :],
                              z_in[:, :].rearrange("(p t) d -> p t d", p=128))

            # ---- phase A: replicated segment stats for all classes ---------
            Mh = pool.tile([128, NT, 2 * D], FH)   # [z | z^2] in fp16
            nc.vector.tensor_copy(Mh[:, :, 0:D], Mf[:, :, :])
            nc.vector.tensor_tensor(Mh[:, :, D:2 * D], Mf[:, :, :],
                                    Mf[:, :, :], ALU.mult)

            # one-hot over all 1000 classes (fp16 exact for labels < 2048);
            # all on DVE: gpsimd shares SBUF ports with DVE
            oh = pool.tile([128, NT, C], FH)
            for t in range(NT):
                nc.vector.tensor_scalar(oh[:, t, :], cv[:], yc[:, t:t + 1],
                                        None, ALU.is_equal)

            # psSS = [S1^T; S2^T]  (128 rows = 2*64 dims, 1000 classes);
            # two 500-col halves: a matmul output cannot span PSUM banks
            psh0 = ppS.tile([128, CH], FP, tag="pss0")
            psh1 = ppS.tile([128, CH], FP, tag="pss1")
            psh = [psh0, psh1]
            for t in range(NT):
                for h in range(2):
                    nc.tensor.matmul(psh[h][:], lhsT=Mh[:, t, :],
                                     rhs=oh[:, t, h * CH:(h + 1) * CH],
                                     start=(t == 0), stop=(t == NT - 1))

            # counts row: ones^T @ (sum of one-hot tiles)
            ohsum = pool.tile([128, C], FH)
            nc.vector.tensor_tensor(ohsum[:], oh[:, 0, :], oh[:, 1, :],
                                    ALU.add)
            for t in range(2, NT):
                nc.vector.tensor_tensor(ohsum[:], ohsum[:], oh[:, t, :],
                                        ALU.add)
            onesh = pool.tile([128, 1], FH)
            nc.vector.memset(onesh[:], 1.0)
            cnt = pool.tile([1, C], FP)            # counts + eps
            for h in range(2):
                psC = pp.tile([1, CH], FP, tag="ps")
                nc.tensor.matmul(psC[:], lhsT=onesh[:],
                                 rhs=ohsum[:, h * CH:(h + 1) * CH],
                                 start=True, stop=True)
                nc.vector.tensor_scalar_add(cnt[:, h * CH:(h + 1) * CH],
                                            psC[:], EPS_STATS)

            # per-class rows: 1/cnt, log prior, correction g = (cnt+2e)/cnt
            # (applied to S1T*meanT, which already carries one 1/cnt)
            rcp = pool.tile([1, C], FP)
            nc.vector.reciprocal(rcp[:], cnt[:])
            beta = pool.tile([1, C], FP)
            nc.scalar.activation(beta[:], cnt[:], AF.Ln, scale=1.0 / TSUM)
            grow = pool.tile([1, C], FP)
            nc.vector.tensor_scalar_add(grow[:], cnt[:], EPS_STATS)
            nc.vector.tensor_tensor(grow[:], grow[:], rcp[:], ALU.mult)

            # stats to SBUF (engines may read at most one PSUM input)
            S = pool.tile([128, C], FP)
            for h in range(2):
                nc.scalar.copy(S[:, h * CH:(h + 1) * CH], psh[h][:])

            # broadcast the rows across partitions (ones^T @ row)
            onesr = pool.tile([1, 128], FP)
            nc.vector.memset(onesr[:], 1.0)
            meanT = pool.tile([64, C], FP)          # = msb of the score phase
            corr = pool.tile([64, C], FP)           # S1T*meanT*g
            for h in range(2):
                hs = slice(h * CH, (h + 1) * CH)
                psRB = pp.tile([64, CH], FP, tag="ps")
                nc.tensor.matmul(psRB[:], lhsT=onesr[:, 0:64], rhs=rcp[:, hs],
                                 start=True, stop=True)
                nc.vector.tensor_tensor(meanT[:, hs], S[0:64, hs],
                                        psRB[:], ALU.mult)
                psGB = pp.tile([64, CH], FP, tag="ps")
                nc.tensor.matmul(psGB[:], lhsT=onesr[:, 0:64], rhs=grow[:, hs],
                                 start=True, stop=True)
                nc.vector.tensor_tensor(corr[:, hs], S[0:64, hs],
                                        meanT[:, hs], ALU.mult)
                nc.vector.tensor_tensor(corr[:, hs], corr[:, hs],
                                        psGB[:], ALU.mult)

            # pooled covariance: (sum_c S2T - sum_c S1T*meanT*g)/TSUM + eps
            s2s = pool.tile([64, 1], FP)
            nc.vector.reduce_sum(s2s[:], S[64:128, :],
                                 axis=mybir.AxisListType.X)
            bsum = pool.tile([64, 1], FP)
            nc.vector.reduce_sum(bsum[:], corr[:, :],
                                 axis=mybir.AxisListType.X)
            pooled = pool.tile([64, 1], FP)
            nc.vector.tensor_tensor(pooled[:], s2s[:], bsum[:], ALU.subtract)
            nc.vector.tensor_scalar(pooled[:], pooled[:], 1.0 / TSUM,
                                    EPS_STATS, ALU.mult, ALU.add)
            pmax = pool.tile([64, 1], FP)
            nc.vector.tensor_scalar_max(pmax[:], pooled[:], EPS_PREC)
            prec = pool.tile([64, 1], FP)
            nc.vector.reciprocal(prec[:], pmax[:])

            # ---- phase B: batch-sharded Mahalanobis scores ------------------
            # local z^T for the score matmuls
            zTq = pool.tile([65, 256], FP)
            nc.vector.memset(zTq[64:65, :], 1.0)
            for j in range(JT):
                psZ = pp.tile([64, 128], FP, tag="ps")
                nc.tensor.transpose(psZ[:], zl[:, j, :], idn[:, :])
                nc.scalar.copy(zTq[0:64, j * 128:(j + 1) * 128], psZ[:])
            zsq = pool.tile([128, JT, D], FP)
            nc.scalar.activation(zsq[:], zl[:], AF.Square)

            msq = pool.tile([64, C], FP)
            nc.scalar.activation(msq[:], meanT[:], AF.Square)

            # PE warm-up (HAM needs a busy stretch before the score matmuls)
            junkps = pp.tile([64, 64], FP, tag="ps")
            for w in range(10):
                nc.tensor.matmul(junkps[:], lhsT=idn[0:64, 0:64],
                                 rhs=meanT[:, w * 64:w * 64 + 64],
                                 start=True, stop=True)

            # prec broadcast across partitions for the q computation
            psPR = pp.tile([1, 64], FP, tag="ps")
            nc.tensor.transpose(psPR[:], prec[:], idn[0:64, 0:64])
            prow = pool.tile([1, 64], FP)
            nc.vector.tensor_copy(prow[:], psPR[:])
            psPB = pp.tile([128, 64], FP, tag="ps")
            nc.tensor.matmul(psPB[:], lhsT=onesr[:], rhs=prow[:],
                             start=True, stop=True)
            precbc = pool.tile([128, 64], FP)
            nc.vector.tensor_copy(precbc[:], psPB[:])

            V = pool.tile([65, C], FP)              # [prec*mean; beta - r/2]
            nc.vector.tensor_scalar(V[0:64, :], meanT[:], prec[:], None,
                                    ALU.mult)
            rrow = pool.tile([1, C], FP)
            for h in range(2):
                psR = pp.tile([1, CH], FP, tag="ps")
                nc.tensor.matmul(psR[:], lhsT=prec[:],
                                 rhs=msq[:, h * CH:(h + 1) * CH],
                                 start=True, stop=True)
                nc.scalar.activation(rrow[:, h * CH:(h + 1) * CH], psR[:],
                                     AF.Copy, scale=-0.5)
            nc.vector.tensor_tensor(rrow[:], rrow[:], beta[:], ALU.add)
            nc.scalar.copy(V[64:65, :], rrow[:])

            junk = pool.tile([128, 64], FP)
            qraw = pool.tile([128, JT], FP)
            qsb = pool.tile([128, JT], FP)
            for j in range(JT):
                nc.vector.tensor_tensor(junk[:], zsq[:, j, :], precbc[:],
                                        ALU.mult)
                nc.vector.reduce_sum(qraw[:, j:j + 1], junk[:],
                                     axis=mybir.AxisListType.X)
            nc.vector.tensor_scalar_mul(qsb[:], qraw[:], -0.5)

            for j in range(JT):
                outj = pool.tile([128, C], FP, tag=f"outsb{j}")
                for h in range(2):
                    psO = pp.tile([128, CH], FP, tag="ps")
                    nc.tensor.matmul(psO[:],
                                     lhsT=zTq[:, j * 128:(j + 1) * 128],
                                     rhs=V[:, h * CH:(h + 1) * CH],
                                     start=True, stop=True)
                    nc.scalar.activation(outj[:, h * CH:(h + 1) * CH],
                                         psO[:], AF.Identity,
                                         bias=qsb[:, j:j + 1], scale=1.0)
                nc.sync.dma_start(out[j * 128:(j + 1) * 128, :], outj[:])

    nc.compile()
    return nc


_NC_CACHE = None


def _get_program():
    global _NC_CACHE
    if _NC_CACHE is None:
        _NC_CACHE = build_program()
    return _NC_CACHE


def make_in_maps(z, y):
    z = np.ascontiguousarray(np.asarray(z, dtype=np.float32))
    yf = np.asarray(y).astype(np.float32)          # labels < 1000, exact
    ycols_np = np.ascontiguousarray(yf.reshape(128, NT))
    cvals_np = np.ascontiguousarray(
        np.broadcast_to(np.arange(C, dtype=np.float16), (128, C)))
    ident_np = np.eye(128, dtype=np.float32)
    in_maps = []
    for k in range(NCORES):
        zloc_np = np.ascontiguousarray(
            z[k * BL:(k + 1) * BL].reshape(JT, 128, D).transpose(1, 0, 2))
        in_maps.append({
            "z_in": z,
            "ycols": ycols_np,
            "cvals": cvals_np,
            "zloc": zloc_np,
            "ident": ident_np,
        })
    return in_maps


def run(z, y, trace=False, **kwargs):
    nc = _get_program()
    res = run_bass_kernel_spmd(nc, make_in_maps(z, y), list(range(NCORES)),
                               trace=trace, **kwargs)
    full = np.concatenate([res.results[k]["out_loc"] for k in range(NCORES)],
                          axis=0)
    return full, res


def kernel(z, y):
    full, _ = run(z, y, trace=False)
    return full


if __name__ == "__main__":
    rng = np.random.default_rng(0)
    z = rng.standard_normal((B, D), dtype=np.float32)
    y = rng.integers(0, C, size=(B,)).astype(np.int64)
    out = kernel(z, y)
    print("out", out.shape, out.dtype, out[0, :4])


# revision 15
# speedup vs baseline: 3.6192x; 1.2234x over previous
"""LDA head (segment-reduce + Mahalanobis scores) on 8 Trainium2 NeuronCores.

Strategy (single SPMD NEFF on 8 cores, fully replicated stats — no
cross-core communication):
  - Every core reads the full batch and computes the segment stats for ALL
    1000 classes:  psSS = M^T @ onehot  with M = [z | z^2] in fp16, i.e. 16
    accumulating PE matmuls (one per 128-row batch tile) producing
    [S1^T; S2^T] = (128, 1000) fp32 in PSUM — already in the d-partition
    layout the score phase needs, so no transposes and no collective at all.
    (The previous designs exchanged class-sharded stats via a CC AllGather /
    remote SBUF DMAs; both cost 50-150us in collective machinery and launch-
    skew barrier waits.  Recomputing on every core costs ~15us of fp16 PE.)
  - Class counts come from a ones^T @ ohsum matmul (ohsum = sum of the 16
    one-hot tiles on DVE).  fp16 is exact for the one-hot compare (labels
    < 2048) and for the counts; z in fp16 only affects mean/var by ~1e-3
    relative, far inside the 2e-2 gate.
  - Per-column (per-class) scaling rows (1/counts, count correction) are
    broadcast across partitions with ones^T @ row PE matmuls.
  - Scores are batch-sharded: core k computes output rows [256k, 256k+256) as
    out = [z^T; 1]^T @ [prec*mean; log prior - 0.5*r]  (+ -0.5*q_b fused into
    the PSUM->SBUF copy as a per-partition activation bias), with z in fp32.

kernel(z, y) takes the full inputs and returns the full (2048, 1000) output.
"""

import sys
import numpy as np

if "/opt/trn_rl_repo" not in sys.path:
    sys.path.insert(0, "/opt/trn_rl_repo")

import concourse.bacc as bacc
import concourse.bass as bass
import concourse.mybir as mybir
from concourse import tile
from concourse.bass_utils import run_bass_kernel_spmd

B, C, D = 2048, 1000, 64
NCORES = 8
NT = B // 128               # 16 batch tiles
BL = B // NCORES            # 256 output rows per core
JT = BL // 128              # 2 local batch tiles
CH = C // 2                 # 500-column halves (PSUM bank = 2KB)
EPS_STATS = 1e-5
EPS_PREC = 1e-6
TSUM = float(np.float32(B) + np.float32(C * EPS_STATS))   # counts.sum()
FP = mybir.dt.float32
FH = mybir.dt.float16
AF = mybir.ActivationFunctionType
ALU = mybir.AluOpType


def build_program():
    nc = bacc.Bacc("TRN2", target_bir_lowering=False, debug=False,
                   num_devices=NCORES)

    z_in = nc.dram_tensor("z_in", [B, D], FP, kind="ExternalInput")
    ycols = nc.dram_tensor("ycols", [128, NT], FP, kind="ExternalInput")
    cvals = nc.dram_tensor("cvals", [128, C], FH, kind="ExternalInput")
    zloc = nc.dram_tensor("zloc", [128, JT, D], FP, kind="ExternalInput")
    ident = nc.dram_tensor("ident", [128, 128], FP, kind="ExternalInput")
    out = nc.dram_tensor("out_loc", [BL, C], FP, kind="ExternalOutput")

    with tile.TileContext(nc) as tc:
        with tc.tile_pool(name="sb", bufs=1) as pool, \
             tc.tile_pool(name="pss", bufs=1, space="PSUM") as ppS, \
             tc.tile_pool(name="ps", bufs=6, space="PSUM") as pp:

            # ---- input DMAs -------------------------------------------------
            cv = pool.tile([128, C], FH)
            nc.sync.dma_start(cv[:], cvals[:, :])
            yc = pool.tile([128, NT], FP)
            nc.sync.dma_start(yc[:], ycols[:, :])
            idn = pool.tile([128, 128], FP)
            nc.sync.dma_start(idn[:], ident[:, :])
            zl = pool.tile([128, JT, D], FP)
            nc.sync.dma_start(zl[:], zloc[:, :, :])
            Mf = pool.tile([128, NT, D], FP)
            # batch row p*NT+t -> partition p: one contiguous 4KB run/partition
            nc.sync.dma_start(Mf[:, :, :],
                              z_in[:, :].rearrange("(p t) d -> p t d", p=128))

            # ---- phase A: replicated segment stats for all classes ---------
            Mh = pool.tile([128, NT, D + 1], FH)   # [z | 1] in fp16
            nc.vector.tensor_copy(Mh[:, :, 0:D], Mf[:, :, :])
            nc.vector.memset(Mh[:, :, D:D + 1], 1.0)

            # one-hot over all 1000 classes (fp16 exact for labels < 2048);
            # all on DVE: gpsimd shares SBUF ports with DVE
            oh = pool.tile([128, NT, C], FH)
            for t in range(NT):
                nc.vector.tensor_scalar(oh[:, t, :], cv[:], yc[:, t:t + 1],
                                        None, ALU.is_equal)

            # psSS = [S1^T; counts]  (65 rows, 1000 classes);
            # two 500-col halves: a matmul output cannot span PSUM banks
            psh0 = ppS.tile([65, CH], FP, tag="pss0")
            psh1 = ppS.tile([65, CH], FP, tag="pss1")
            psh = [psh0, psh1]
            for t in range(NT):
                for h in range(2):
                    nc.tensor.matmul(psh[h][:], lhsT=Mh[:, t, :],
                                     rhs=oh[:, t, h * CH:(h + 1) * CH],
                                     start=(t == 0), stop=(t == NT - 1))

            # sum_b z^2 over the full batch (class-independent: every batch
            # row lands in exactly one class, so sum_c S2T[d,c] = sum_b z^2)
            sq32 = pool.tile([128, NT, D], FP)
            nc.scalar.activation(sq32[:], Mf[:], AF.Square)
            zs2 = pool.tile([128, D], FP)
            nc.vector.reduce_sum(zs2[:], sq32[:].rearrange("p t d -> p d t"),
                                 axis=mybir.AxisListType.X)
            ones128 = pool.tile([128, 1], FP)
            nc.vector.memset(ones128[:], 1.0)
            psS2 = pp.tile([D, 1], FP, tag="ps")
            nc.tensor.matmul(psS2[:], lhsT=zs2[:], rhs=ones128[:],
                             start=True, stop=True)
            s2s = pool.tile([64, 1], FP)
            nc.vector.tensor_copy(s2s[:], psS2[:])

            # stats to SBUF (engines may read at most one PSUM input)
            S = pool.tile([65, C], FP)
            for h in range(2):
                nc.scalar.copy(S[:, h * CH:(h + 1) * CH], psh[h][:])

            # per-class rows: counts, 1/cnt, log prior, g = (cnt+2e)/cnt
            # (applied to S1T*meanT, which already carries one 1/cnt);
            # reciprocal on the scalar engine: ~6x faster per element than DVE
            cnt = pool.tile([1, C], FP)            # counts + eps
            nc.vector.tensor_scalar_add(cnt[:], S[64:65, :], EPS_STATS)
            lncnt = pool.tile([1, C], FP)
            nc.scalar.activation(lncnt[:], cnt[:], AF.Ln)
            rcp = pool.tile([1, C], FP)            # 1/cnt = exp(-ln cnt)
            nc.scalar.activation(rcp[:], lncnt[:], AF.Exp, scale=-1.0)
            beta = pool.tile([1, C], FP)           # ln(cnt/TSUM)
            nc.vector.tensor_scalar_add(beta[:], lncnt[:],
                                        -float(np.log(TSUM)))
            grow = pool.tile([1, C], FP)
            nc.vector.tensor_scalar_add(grow[:], cnt[:], EPS_STATS)
            nc.vector.tensor_tensor(grow[:], grow[:], rcp[:], ALU.mult)

            # broadcast the rows across partitions (ones^T @ row)
            onesr = pool.tile([1, 128], FP)
            nc.vector.memset(onesr[:], 1.0)
            meanT = pool.tile([64, C], FP)          # = msb of the score phase
            corr = pool.tile([64, C], FP)           # S1T*meanT*g
            for h in range(2):
                hs = slice(h * CH, (h + 1) * CH)
                psRB = pp.tile([64, CH], FP, tag="ps")
                nc.tensor.matmul(psRB[:], lhsT=onesr[:, 0:64], rhs=rcp[:, hs],
                                 start=True, stop=True)
                nc.vector.tensor_tensor(meanT[:, hs], S[0:64, hs],
                                        psRB[:], ALU.mult)
                psGB = pp.tile([64, CH], FP, tag="ps")
                nc.tensor.matmul(psGB[:], lhsT=onesr[:, 0:64], rhs=grow[:, hs],
                                 start=True, stop=True)
                nc.vector.tensor_tensor(corr[:, hs], S[0:64, hs],
                                        meanT[:, hs], ALU.mult)
                nc.vector.tensor_tensor(corr[:, hs], corr[:, hs],
                                        psGB[:], ALU.mult)

            # pooled covariance: (sum_b z^2 - sum_c S1T*meanT*g)/TSUM + eps
            bsum = pool.tile([64, 1], FP)
            nc.vector.reduce_sum(bsum[:], corr[:, :],
                                 axis=mybir.AxisListType.X)
            pooled = pool.tile([64, 1], FP)
            nc.vector.tensor_tensor(pooled[:], s2s[:], bsum[:], ALU.subtract)
            nc.vector.tensor_scalar(pooled[:], pooled[:], 1.0 / TSUM,
                                    EPS_STATS, ALU.mult, ALU.add)
            pmax = pool.tile([64, 1], FP)
            nc.vector.tensor_scalar_max(pmax[:], pooled[:], EPS_PREC)
            prec = pool.tile([64, 1], FP)
            nc.vector.reciprocal(prec[:], pmax[:])

            # ---- phase B: batch-sharded Mahalanobis scores ------------------
            # local z^T for the score matmuls
            zTq = pool.tile([65, 256], FH)
            nc.vector.memset(zTq[64:65, :], 1.0)
            for j in range(JT):
                psZ = pp.tile([64, 128], FP, tag="ps")
                nc.tensor.transpose(psZ[:], zl[:, j, :], idn[:, :])
                nc.scalar.copy(zTq[0:64, j * 128:(j + 1) * 128], psZ[:])
            zsq = pool.tile([128, JT, D], FP)
            nc.scalar.activation(zsq[:], zl[:], AF.Square)

            msq = pool.tile([64, C], FP)
            nc.scalar.activation(msq[:], meanT[:], AF.Square)

            # PE warm-up (HAM needs a busy stretch before the score matmuls)
            junkps = pp.tile([64, 64], FP, tag="ps")
            for w in range(10):
                nc.tensor.matmul(junkps[:], lhsT=idn[0:64, 0:64],
                                 rhs=meanT[:, w * 64:w * 64 + 64],
                                 start=True, stop=True)

            # prec broadcast across partitions for the q computation
            psPR = pp.tile([1, 64], FP, tag="ps")
            nc.tensor.transpose(psPR[:], prec[:], idn[0:64, 0:64])
            prow = pool.tile([1, 64], FP)
            nc.vector.tensor_copy(prow[:], psPR[:])
            psPB = pp.tile([128, 64], FP, tag="ps")
            nc.tensor.matmul(psPB[:], lhsT=onesr[:], rhs=prow[:],
                             start=True, stop=True)
            precbc = pool.tile([128, 64], FP)
            nc.vector.tensor_copy(precbc[:], psPB[:])

            V = pool.tile([65, C], FH)              # [prec*mean; beta - r/2]
            nc.vector.tensor_scalar(V[0:64, :], meanT[:], prec[:], None,
                                    ALU.mult)
            rrow = pool.tile([1, C], FP)
            for h in range(2):
                psR = pp.tile([1, CH], FP, tag="ps")
                nc.tensor.matmul(psR[:], lhsT=prec[:],
                                 rhs=msq[:, h * CH:(h + 1) * CH],
                                 start=True, stop=True)
                nc.scalar.activation(rrow[:, h * CH:(h + 1) * CH], psR[:],
                                     AF.Copy, scale=-0.5)
            nc.vector.tensor_tensor(rrow[:], rrow[:], beta[:], ALU.add)
            nc.scalar.copy(V[64:65, :], rrow[:])

            junk = pool.tile([128, 64], FP)
            qraw = pool.tile([128, JT], FP)
            qsb = pool.tile([128, JT], FP)
            for j in range(JT):
                nc.vector.tensor_tensor(junk[:], zsq[:, j, :], precbc[:],
                                        ALU.mult)
                nc.vector.reduce_sum(qraw[:, j:j + 1], junk[:],
                                     axis=mybir.AxisListType.X)
            nc.vector.tensor_scalar_mul(qsb[:], qraw[:], -0.5)

            for j in range(JT):
                outj = pool.tile([128, C], FP, tag=f"outsb{j}")
                for h in range(2):
                    psO = pp.tile([128, CH], FP, tag="ps")
                    nc.tensor.matmul(psO[:],
                                     lhsT=zTq[:, j * 128:(j + 1) * 128],
                                     rhs=V[:, h * CH:(h + 1) * CH],
                                     start=True, stop=True)
                    nc.scalar.activation(outj[:, h * CH:(h + 1) * CH],
                                         psO[:], AF.Identity,
                                         bias=qsb[:, j:j + 1], scale=1.0)
                nc.sync.dma_start(out[j * 128:(j + 1) * 128, :], outj[:])

    nc.compile()
    return nc


_NC_CACHE = None


def _get_program():
    global _NC_CACHE
    if _NC_CACHE is None:
        _NC_CACHE = build_program()
    return _NC_CACHE


def make_in_maps(z, y):
    z = np.ascontiguousarray(np.asarray(z, dtype=np.float32))
    yf = np.asarray(y).astype(np.float32)          # labels < 1000, exact
    ycols_np = np.ascontiguousarray(yf.reshape(128, NT))
    cvals_np = np.ascontiguousarray(
        np.broadcast_to(np.arange(C, dtype=np.float16), (128, C)))
    ident_np = np.eye(128, dtype=np.float32)
    in_maps = []
    for k in range(NCORES):
        zloc_np = np.ascontiguousarray(
            z[k * BL:(k + 1) * BL].reshape(JT, 128, D).transpose(1, 0, 2))
        in_maps.append({
            "z_in": z,
            "ycols": ycols_np,
            "cvals": cvals_np,
            "zloc": zloc_np,
            "ident": ident_np,
        })
    return in_maps


def run(z, y, trace=False, **kwargs):
    nc = _get_program()
    res = run_bass_kernel_spmd(nc, make_in_maps(z, y), list(range(NCORES)),
                               trace=trace, **kwargs)
    full = np.concatenate([res.results[k]["out_loc"] for k in range(NCORES)],
                          axis=0)
    return full, res


def kernel(z, y):
    full, _ = run(z, y, trace=False)
    return full


if __name__ == "__main__":
    rng = np.random.default_rng(0)
    z = rng.standard_normal((B, D), dtype=np.float32)
    y = rng.integers(0, C, size=(B,)).astype(np.int64)
    out = kernel(z, y)
    print("out", out.shape, out.dtype, out[0, :4])


# revision 16
# speedup vs baseline: 3.7244x; 1.0291x over previous
"""LDA head (segment-reduce + Mahalanobis scores) on 8 Trainium2 NeuronCores.

Strategy (single SPMD NEFF on 8 cores, fully replicated stats — no
cross-core communication):
  - Every core reads the full batch and computes the segment stats for ALL
    1000 classes:  psSS = M^T @ onehot  with M = [z | z^2] in fp16, i.e. 16
    accumulating PE matmuls (one per 128-row batch tile) producing
    [S1^T; S2^T] = (128, 1000) fp32 in PSUM — already in the d-partition
    layout the score phase needs, so no transposes and no collective at all.
    (The previous designs exchanged class-sharded stats via a CC AllGather /
    remote SBUF DMAs; both cost 50-150us in collective machinery and launch-
    skew barrier waits.  Recomputing on every core costs ~15us of fp16 PE.)
  - Class counts come from a ones^T @ ohsum matmul (ohsum = sum of the 16
    one-hot tiles on DVE).  fp16 is exact for the one-hot compare (labels
    < 2048) and for the counts; z in fp16 only affects mean/var by ~1e-3
    relative, far inside the 2e-2 gate.
  - Per-column (per-class) scaling rows (1/counts, count correction) are
    broadcast across partitions with ones^T @ row PE matmuls.
  - Scores are batch-sharded: core k computes output rows [256k, 256k+256) as
    out = [z^T; 1]^T @ [prec*mean; log prior - 0.5*r]  (+ -0.5*q_b fused into
    the PSUM->SBUF copy as a per-partition activation bias), with z in fp32.

kernel(z, y) takes the full inputs and returns the full (2048, 1000) output.
"""

import sys
import numpy as np

if "/opt/trn_rl_repo" not in sys.path:
    sys.path.insert(0, "/opt/trn_rl_repo")

import concourse.bacc as bacc
import concourse.bass as bass
import concourse.mybir as mybir
from concourse import tile
from concourse.bass_utils import run_bass_kernel_spmd

B, C, D = 2048, 1000, 64
NCORES = 8
NT = B // 128               # 16 batch tiles
BL = B // NCORES            # 256 output rows per core
JT = BL // 128              # 2 local batch tiles
CH = C // 2                 # 500-column halves (PSUM bank = 2KB)
EPS_STATS = 1e-5
EPS_PREC = 1e-6
TSUM = float(np.float32(B) + np.float32(C * EPS_STATS))   # counts.sum()
FP = mybir.dt.float32
FH = mybir.dt.float16
AF = mybir.ActivationFunctionType
ALU = mybir.AluOpType


def build_program():
    nc = bacc.Bacc("TRN2", target_bir_lowering=False, debug=False,
                   num_devices=NCORES)

    z_in = nc.dram_tensor("z_in", [B, D], FP, kind="ExternalInput")
    ycols = nc.dram_tensor("ycols", [128, NT], FP, kind="ExternalInput")
    cvals = nc.dram_tensor("cvals", [128, C], FH, kind="ExternalInput")
    zloc = nc.dram_tensor("zloc", [128, JT, D], FP, kind="ExternalInput")
    ident = nc.dram_tensor("ident", [128, 128], FP, kind="ExternalInput")
    out = nc.dram_tensor("out_loc", [BL, C], FP, kind="ExternalOutput")

    with tile.TileContext(nc) as tc:
        with tc.tile_pool(name="sb", bufs=1) as pool, \
             tc.tile_pool(name="pss", bufs=1, space="PSUM") as ppS, \
             tc.tile_pool(name="ps", bufs=6, space="PSUM") as pp:

            # ---- input DMAs -------------------------------------------------
            cv = pool.tile([128, C], FH)
            nc.sync.dma_start(cv[:], cvals[:, :])
            yc = pool.tile([128, NT], FP)
            nc.sync.dma_start(yc[:], ycols[:, :])
            idn = pool.tile([128, 128], FP)
            nc.sync.dma_start(idn[:], ident[:, :])
            zl = pool.tile([128, JT, D], FP)
            nc.sync.dma_start(zl[:], zloc[:, :, :])
            Mf = pool.tile([128, NT, D], FP)
            # batch row p*NT+t -> partition p: one contiguous 4KB run/partition
            nc.sync.dma_start(Mf[:, :, :],
                              z_in[:, :].rearrange("(p t) d -> p t d", p=128))

            # ---- phase A: replicated segment stats for all classes ---------
            Mh = pool.tile([128, NT, D + 1], FH)   # [z | 1] in fp16
            nc.vector.tensor_copy(Mh[:, :, 0:D], Mf[:, :, :])
            nc.vector.memset(Mh[:, :, D:D + 1], 1.0)

            # one-hot over all 1000 classes (fp16 exact for labels < 2048);
            # all on DVE: gpsimd shares SBUF ports with DVE
            oh = pool.tile([128, NT, C], FH)
            for t in range(NT):
                nc.vector.tensor_scalar(oh[:, t, :], cv[:], yc[:, t:t + 1],
                                        None, ALU.is_equal)

            # psSS = [S1^T; counts]  (65 rows, 1000 classes);
            # two 500-col halves: a matmul output cannot span PSUM banks
            psh0 = ppS.tile([65, CH], FP, tag="pss0")
            psh1 = ppS.tile([65, CH], FP, tag="pss1")
            psh = [psh0, psh1]
            for t in range(NT):
                for h in range(2):
                    nc.tensor.matmul(psh[h][:], lhsT=Mh[:, t, :],
                                     rhs=oh[:, t, h * CH:(h + 1) * CH],
                                     start=(t == 0), stop=(t == NT - 1))

            # sum_b z^2 over the full batch (class-independent: every batch
            # row lands in exactly one class, so sum_c S2T[d,c] = sum_b z^2)
            sq32 = pool.tile([128, NT, D], FP)
            nc.scalar.activation(sq32[:], Mf[:], AF.Square)
            zs2 = pool.tile([128, D], FP)
            nc.vector.reduce_sum(zs2[:], sq32[:].rearrange("p t d -> p d t"),
                                 axis=mybir.AxisListType.X)
            ones128 = pool.tile([128, 1], FP)
            nc.vector.memset(ones128[:], 1.0)
            psS2 = pp.tile([D, 1], FP, tag="ps")
            nc.tensor.matmul(psS2[:], lhsT=zs2[:], rhs=ones128[:],
                             start=True, stop=True)
            s2s = pool.tile([64, 1], FP)
            nc.vector.tensor_copy(s2s[:], psS2[:])

            # stats to SBUF (engines may read at most one PSUM input)
            S = pool.tile([65, C], FP)
            for h in range(2):
                nc.scalar.copy(S[:, h * CH:(h + 1) * CH], psh[h][:])

            # per-class rows: counts, 1/cnt, log prior, g = (cnt+2e)/cnt
            # (applied to S1T*meanT, which already carries one 1/cnt);
            # reciprocal on the scalar engine: ~6x faster per element than DVE
            cnt = pool.tile([1, C], FP)            # counts + eps
            nc.vector.tensor_scalar_add(cnt[:], S[64:65, :], EPS_STATS)
            lncnt = pool.tile([1, C], FP)
            nc.scalar.activation(lncnt[:], cnt[:], AF.Ln)
            rcp = pool.tile([1, C], FP)            # 1/cnt = exp(-ln cnt)
            nc.scalar.activation(rcp[:], lncnt[:], AF.Exp, scale=-1.0)
            beta = pool.tile([1, C], FP)           # ln(cnt/TSUM)
            nc.vector.tensor_scalar_add(beta[:], lncnt[:],
                                        -float(np.log(TSUM)))
            cnt2 = pool.tile([1, C], FP)           # cnt + 2*eps
            nc.vector.tensor_scalar_add(cnt2[:], cnt[:], EPS_STATS)

            # broadcast the rows across partitions (ones^T @ row); the cnt2
            # broadcast has no Ln/Exp dependency so it overlaps the rcp chain,
            # and meanT^2 doubles as the score phase's msq
            onesr = pool.tile([1, 128], FP)
            nc.vector.memset(onesr[:], 1.0)
            meanT = pool.tile([64, C], FP)          # = msb of the score phase
            msq = pool.tile([64, C], FP)            # meanT^2
            corr = pool.tile([64, C], FP)           # meanT^2*(cnt+2e)
            for h in range(2):
                hs = slice(h * CH, (h + 1) * CH)
                psGB = pp.tile([64, CH], FP, tag="ps")
                nc.tensor.matmul(psGB[:], lhsT=onesr[:, 0:64], rhs=cnt2[:, hs],
                                 start=True, stop=True)
                psRB = pp.tile([64, CH], FP, tag="ps")
                nc.tensor.matmul(psRB[:], lhsT=onesr[:, 0:64], rhs=rcp[:, hs],
                                 start=True, stop=True)
                nc.vector.tensor_tensor(meanT[:, hs], S[0:64, hs],
                                        psRB[:], ALU.mult)
                nc.vector.tensor_tensor(msq[:, hs], meanT[:, hs],
                                        meanT[:, hs], ALU.mult)
                nc.vector.tensor_tensor(corr[:, hs], msq[:, hs],
                                        psGB[:], ALU.mult)

            # pooled covariance: (sum_b z^2 - sum_c S1T*meanT*g)/TSUM + eps
            bsum = pool.tile([64, 1], FP)
            nc.vector.reduce_sum(bsum[:], corr[:, :],
                                 axis=mybir.AxisListType.X)
            pooled = pool.tile([64, 1], FP)
            nc.vector.tensor_tensor(pooled[:], s2s[:], bsum[:], ALU.subtract)
            nc.vector.tensor_scalar(pooled[:], pooled[:], 1.0 / TSUM,
                                    EPS_STATS, ALU.mult, ALU.add)
            pmax = pool.tile([64, 1], FP)
            nc.vector.tensor_scalar_max(pmax[:], pooled[:], EPS_PREC)
            prec = pool.tile([64, 1], FP)
            nc.vector.reciprocal(prec[:], pmax[:])

            # ---- phase B: batch-sharded Mahalanobis scores ------------------
            # local z^T for the score matmuls
            zTq = pool.tile([65, 256], FH)
            nc.vector.memset(zTq[64:65, :], 1.0)
            for j in range(JT):
                psZ = pp.tile([64, 128], FP, tag="ps")
                nc.tensor.transpose(psZ[:], zl[:, j, :], idn[:, :])
                nc.scalar.copy(zTq[0:64, j * 128:(j + 1) * 128], psZ[:])
            zsq = pool.tile([128, JT, D], FP)
            nc.scalar.activation(zsq[:], zl[:], AF.Square)

            # PE warm-up (HAM needs a busy stretch before the score matmuls)
            junkps = pp.tile([64, 64], FP, tag="ps")
            for w in range(10):
                nc.tensor.matmul(junkps[:], lhsT=idn[0:64, 0:64],
                                 rhs=meanT[:, w * 64:w * 64 + 64],
                                 start=True, stop=True)

            # prec broadcast across partitions for the q computation
            psPR = pp.tile([1, 64], FP, tag="ps")
            nc.tensor.transpose(psPR[:], prec[:], idn[0:64, 0:64])
            prow = pool.tile([1, 64], FP)
            nc.vector.tensor_copy(prow[:], psPR[:])
            psPB = pp.tile([128, 64], FP, tag="ps")
            nc.tensor.matmul(psPB[:], lhsT=onesr[:], rhs=prow[:],
                             start=True, stop=True)
            precbc = pool.tile([128, 64], FP)
            nc.vector.tensor_copy(precbc[:], psPB[:])

            V = pool.tile([65, C], FH)              # [prec*mean; beta - r/2]
            nc.vector.tensor_scalar(V[0:64, :], meanT[:], prec[:], None,
                                    ALU.mult)
            rrow = pool.tile([1, C], FP)
            for h in range(2):
                psR = pp.tile([1, CH], FP, tag="ps")
                nc.tensor.matmul(psR[:], lhsT=prec[:],
                                 rhs=msq[:, h * CH:(h + 1) * CH],
                                 start=True, stop=True)
                nc.scalar.activation(rrow[:, h * CH:(h + 1) * CH], psR[:],
                                     AF.Copy, scale=-0.5)
            nc.vector.tensor_tensor(rrow[:], rrow[:], beta[:], ALU.add)
            nc.scalar.copy(V[64:65, :], rrow[:])

            junk = pool.tile([128, 64], FP)
            qraw = pool.tile([128, JT], FP)
            qsb = pool.tile([128, JT], FP)
            for j in range(JT):
                nc.vector.tensor_tensor(junk[:], zsq[:, j, :], precbc[:],
                                        ALU.mult)
                nc.vector.reduce_sum(qraw[:, j:j + 1], junk[:],
                                     axis=mybir.AxisListType.X)
            nc.vector.tensor_scalar_mul(qsb[:], qraw[:], -0.5)

            for j in range(JT):
                outj = pool.tile([128, C], FP, tag=f"outsb{j}")
                for h in range(2):
                    psO = pp.tile([128, CH], FP, tag="ps")
                    nc.tensor.matmul(psO[:],
                                     lhsT=zTq[:, j * 128:(j + 1) * 128],
                                     rhs=V[:, h * CH:(h + 1) * CH],
                                     start=True, stop=True)
                    nc.scalar.activation(outj[:, h * CH:(h + 1) * CH],
                                         psO[:], AF.Identity,
                                         bias=qsb[:, j:j + 1], scale=1.0)
                nc.sync.dma_start(out[j * 128:(j + 1) * 128, :], outj[:])

    nc.compile()
    return nc


_NC_CACHE = None


def _get_program():
    global _NC_CACHE
    if _NC_CACHE is None:
        _NC_CACHE = build_program()
    return _NC_CACHE


def make_in_maps(z, y):
    z = np.ascontiguousarray(np.asarray(z, dtype=np.float32))
    yf = np.asarray(y).astype(np.float32)          # labels < 1000, exact
    ycols_np = np.ascontiguousarray(yf.reshape(128, NT))
    cvals_np = np.ascontiguousarray(
        np.broadcast_to(np.arange(C, dtype=np.float16), (128, C)))
    ident_np = np.eye(128, dtype=np.float32)
    in_maps = []
    for k in range(NCORES):
        zloc_np = np.ascontiguousarray(
            z[k * BL:(k + 1) * BL].reshape(JT, 128, D).transpose(1, 0, 2))
        in_maps.append({
            "z_in": z,
            "ycols": ycols_np,
            "cvals": cvals_np,
            "zloc": zloc_np,
            "ident": ident_np,
        })
    return in_maps


def run(z, y, trace=False, **kwargs):
    nc = _get_program()
    res = run_bass_kernel_spmd(nc, make_in_maps(z, y), list(range(NCORES)),
                               trace=trace, **kwargs)
    full = np.concatenate([res.results[k]["out_loc"] for k in range(NCORES)],
                          axis=0)
    return full, res


def kernel(z, y):
    full, _ = run(z, y, trace=False)
    return full


if __name__ == "__main__":
    rng = np.random.default_rng(0)
    z = rng.standard_normal((B, D), dtype=np.float32)
    y = rng.integers(0, C, size=(B,)).astype(np.int64)
    out = kernel(z, y)
    print("out", out.shape, out.dtype, out[0, :4])
